# Initial kernel scaffold
#
"""SimpleGCN (3-layer GCNConv + global_add_pool + linear head) on 8 Trainium2 cores.

Strategy (self-contained; shapes hardcoded for the nn_SimpleGCN problem):
 - Nodes sharded contiguously across 8 cores by dst (12500 each).
 - Per layer, per core: t' = (h @ W) * dinv[node] for the local shard (PE),
   written bf16 (256B rows); broadcast via FOUR pipelined AllGathers, one per
   "pool" (pool q = concat over cores of their q-th 3125-row sub-slice), so
   gathers for pool q start as soon as AG_q lands.
 - Message aggregation: edges (self-loops EXCLUDED) sorted per (dst-block,
   src-pool), densely packed into 128-edge chunks (chunks may span two dst
   blocks; each (chunk, block) pair gets its own one-hot S slot built with
   is_equal), dma_gather pulls t'[src] rows (int16 pool-local indices),
   PE matmul-accumulates per-128-dst-block segment sums in PSUM.
 - Self-loop contribution added per block via identity-matmul of the local
   t' block (DMA'd back from agin) into the same PSUM group.
 - h' = relu(dinv*psum + bias); transposed (PE) back to feat-major for next
   layer. Emission order is software-pipelined: phase A / AG of layer l+1 are
   emitted as soon as the blocks they need are emitted in layer l, so the
   GpSimd gather-descriptor stream (the bottleneck engine) never stalls at
   layer boundaries.
 - Layer 3: per-block pooling matmul into a per-core local-graph window;
   head matmul gives per-core partial logits; host sums partials + head_b.
"""
import math
import numpy as np

N_NODES = 100000
N_EDGES = 1600000
D = 128
L = 3
G = 512
NC = 8
SH = N_NODES // NC            # 12500 nodes per core
NBLK = math.ceil(SH / 128)    # 98 blocks (97 full + one of 84)
BW = [128] * (NBLK - 1) + [SH - 128 * (NBLK - 1)]
NQ = 4                        # pools (AllGather pieces)
# every pool must satisfy 8*rows <= 32767 (int16 gather indices).
SLICES = [3125, 3125, 3125, 3125]
SB = [0, 3125, 6250, 9375, 12500]   # per-core pool row boundaries
WCH = 14                      # msg chunks per gather window
MSG_BUFS = 5
S_BUFS = 2
HT_COLS = NBLK * 128          # 12544 (padded node cols)
POOLW = 256                   # per-core local pooled window
STAG = 3                      # warm-start stagger (first windows only)
APART = 14                    # phase-A blocks per part (7 parts)


def _prep(x, edge_index, batch, Ws, bs, head_w, head_b):
    x = np.asarray(x, np.float32)
    ei = np.asarray(edge_index, np.int64)
    batch = np.asarray(batch, np.int64)
    Ws = np.asarray(Ws, np.float32)
    bs = np.asarray(bs, np.float32)
    head_w = np.asarray(head_w, np.float32)

    # degree includes self-loops (as in reference)
    deg = np.bincount(ei[1], minlength=N_NODES).astype(np.float32) + 1.0
    dinv = (1.0 / np.sqrt(deg)).astype(np.float32)

    src_all, dst_all = ei[0], ei[1]

    # ---- per-core edge streams by (dst block, src pool); no self-loops ----
    core = dst_all // SH
    sb_arr = np.asarray(SB[1:], np.int64)
    per_core = []                    # (src_sorted, dloc_sorted, group starts)
    counts = np.zeros((NC, NBLK * NQ), np.int64)
    for c in range(NC):
        m = core == c
        s_c = src_all[m]
        dloc = dst_all[m] - c * SH
        b = dloc >> 7
        q = np.searchsorted(sb_arr, s_c % SH, side="right")   # src pool
        key = b * NQ + q
        order = np.argsort(key, kind="stable")
        counts[c] = np.bincount(key, minlength=NBLK * NQ)
        per_core.append((s_c[order], dloc[order], np.cumsum(counts[c]) - counts[c]))

    cmax = counts.max(axis=0).reshape(NBLK, NQ)     # common group sizes

    # ---- dense slot layout per pool (shared across cores) ----
    # pool q stream: groups (b=0..97, q) concatenated, then chunked at 128.
    grp_base = np.zeros((NBLK, NQ), np.int64)       # slot offset within pool
    pool_slots = np.zeros(NQ, np.int64)
    for q in range(NQ):
        run = 0
        for b in range(NBLK):
            grp_base[b, q] = run
            run += int(cmax[b, q])
        pool_slots[q] = run
    NCHUNK = [int(-(-pool_slots[q] // 128)) for q in range(NQ)]
    NW = [int(-(-NCHUNK[q] // WCH)) for q in range(NQ)]

    # per (q, chunk): list of dst blocks present -> S slots
    # slot table: per (q, w): list of (chunk_in_window, bvar)
    win_slots = {}                                  # (q, w) -> list
    blk_refs = [[] for _ in range(NBLK)]            # b -> (q, w, slot, chunk)
    blk_lastw = np.zeros((NBLK, NQ), np.int64)      # last window of b in q
    for q in range(NQ):
        for b in range(NBLK):
            n = int(cmax[b, q])
            if n == 0:
                continue
            st = int(grp_base[b, q])
            c0, c1 = st // 128, (st + n - 1) // 128
            for ch in range(c0, c1 + 1):
                w = ch // WCH
                key = (q, w)
                slots = win_slots.setdefault(key, [])
                slot = len(slots)
                slots.append((ch % WCH, b))
                blk_refs[b].append((q, w, slot, ch % WCH))
            blk_lastw[b, q] = c1 // WCH
    SMAX = max(len(v) for v in win_slots.values())

    # window metadata (structural, same for all cores)
    win_wch = {}
    win_nidx = {}
    for q in range(NQ):
        for w in range(NW[q]):
            lo, hi = w * WCH, min((w + 1) * WCH, NCHUNK[q])
            win_wch[(q, w)] = hi - lo
            win_nidx[(q, w)] = min((hi - lo) * 128, 10 ** 9)
    # last window of pool q covers chunks up to NCHUNK[q]; all its slots are
    # gathered (pad idx 0) so msg rows are always defined.

    # column layouts
    # idx: per (q, w) wrapped [16, nidx/16] replicated x8 -> [128, nidx/16]
    idx_cols_per = {k: win_nidx[k] // 16 for k in win_nidx}
    # dl: per (q, w): SMAX slot columns
    order_keys = [(q, w) for q in range(NQ) for w in range(NW[q])]
    idx_col_base = {}
    run = 0
    for k in order_keys:
        idx_col_base[k] = run
        run += idx_cols_per[k]
    idx_cols = run
    dl_col_base = {k: i * SMAX for i, k in enumerate(order_keys)}
    dl_cols = len(order_keys) * SMAX

    # block ready rule: window-progress vector (per q) must reach blk_lastw[b]
    # pooled windows
    ws_blk = [max(0, int(b * 128 * G / N_NODES) - 32) for b in range(NBLK)]

    ins_per_core = []
    pooled_base = np.zeros(NC, np.int64)
    for c in range(NC):
        s_c, dloc_c, starts = per_core[c]
        idx_flat = {}                               # (q,) -> pool slot idx array
        dlv = np.full((dl_cols, 128), -1.0, np.float32)   # [slotcol, row]
        ixv = {q: np.zeros(int(-(-pool_slots[q] // 128)) * 128, np.int64)
               for q in range(NQ)}
        dloc_slot = {q: np.full(int(-(-pool_slots[q] // 128)) * 128, -(10 ** 6),
                                np.int64) for q in range(NQ)}
        for q in range(NQ):
            for b in range(NBLK):
                n = int(counts[c][b * NQ + q])
                if n == 0:
                    continue
                st = int(starts[b * NQ + q])
                base = int(grp_base[b, q])
                s_grp = s_c[st:st + n]
                pos = (s_grp // SH) * SLICES[q] + (s_grp % SH) - SB[q]
                ixv[q][base:base + n] = pos
                dloc_slot[q][base:base + n] = dloc_c[st:st + n]
        # dl slot columns
        for (q, w), slots in win_slots.items():
            for slot, (ch, bvar) in enumerate(slots):
                ch_g = w * WCH + ch
                rows = dloc_slot[q][ch_g * 128:(ch_g + 1) * 128]
                col = dl_col_base[(q, w)] + slot
                v = rows.astype(np.float64) - 128.0 * bvar
                v[rows < 0] = -1.0
                dlv[col, :len(rows)] = v.astype(np.float32)
        # idx wrapped layout
        idx_sb = np.zeros((128, idx_cols), np.int64)
        for (q, w) in order_keys:
            nidx = win_nidx[(q, w)]
            base_slot = w * WCH * 128
            vals = ixv[q][base_slot:base_slot + nidx]
            arr = vals.reshape(nidx // 16, 16)       # [j//16, j%16]
            wrapped = arr.T                          # [16, nidx/16]
            rep = np.tile(wrapped, (8, 1))           # [128, nidx/16]
            cb = idx_col_base[(q, w)]
            idx_sb[:, cb:cb + nidx // 16] = rep

        xT = np.zeros((128, HT_COLS), np.float32)
        xT[:, :SH] = x[c * SH:(c + 1) * SH].T
        dinv_c = np.ones((128, NBLK), np.float32)
        dv = dinv[c * SH:(c + 1) * SH]
        for b in range(NBLK):
            dinv_c[:BW[b], b] = dv[b * 128:b * 128 + BW[b]]
        bl = batch[c * SH:(c + 1) * SH]
        g0 = int(bl[0])
        pooled_base[c] = g0
        brel = np.full((128, NBLK), -1.0, np.float32)
        for b in range(NBLK):
            rel = (bl[b * 128:b * 128 + BW[b]] - g0 - ws_blk[b]).astype(np.int64)
            assert rel.min() >= 0 and rel.max() < 128, (c, b, rel.min(), rel.max())
            brel[:BW[b], b] = rel.astype(np.float32)
        iota = np.broadcast_to(np.arange(128, dtype=np.float32), (128, 128)).copy()
        iota3 = np.tile(np.arange(128, dtype=np.float32), (128, SMAX)).copy()
        Wk = np.ascontiguousarray(Ws.transpose(1, 0, 2).reshape(128, L * 128))
        bias_b = np.ascontiguousarray(
            np.broadcast_to(bs[:, None, :], (L, 128, 128)).transpose(1, 0, 2).reshape(128, L * 128))
        import ml_dtypes
        ins_per_core.append({
            "xT": xT, "Wk": Wk, "biasb": bias_b, "dinvc": dinv_c, "brel": brel,
            "iota": iota,
            "iota3": iota3.astype(ml_dtypes.bfloat16),
            "hw": head_w.reshape(128, 1).astype(np.float32),
            "idx": idx_sb.astype(np.int16),
            "dl": np.ascontiguousarray(dlv.T).astype(ml_dtypes.bfloat16),
            "idx0": np.zeros((128, 8), np.int16),
        })
    struct = {
        "bias_zero": bool(np.all(bs == 0.0)),
        "NW": NW, "SMAX": SMAX, "idx_cols": idx_cols, "dl_cols": dl_cols,
        "idx_col_base": idx_col_base, "dl_col_base": dl_col_base,
        "win_slots": win_slots, "win_wch": win_wch, "win_nidx": win_nidx,
        "blk_refs": blk_refs, "blk_lastw": blk_lastw,
        "ws_blk": ws_blk, "pooled_base": pooled_base,
        "head_b": float(np.asarray(head_b).reshape(-1)[0]),
    }
    return ins_per_core, struct


def _build(struct):
    import concourse.bass as bass
    import concourse.bacc as bacc
    import concourse.mybir as mybir
    import concourse.tile as tile
    from concourse.masks import make_identity

    NW = struct["NW"]
    SMAX = struct["SMAX"]
    idx_cols = struct["idx_cols"]
    dl_cols = struct["dl_cols"]
    idx_col_base = struct["idx_col_base"]
    dl_col_base = struct["dl_col_base"]
    win_slots = struct["win_slots"]
    win_wch = struct["win_wch"]
    win_nidx = struct["win_nidx"]
    blk_refs = struct["blk_refs"]
    blk_lastw = struct["blk_lastw"]
    ws_blk = struct["ws_blk"]
    f32 = mybir.dt.float32
    bf16 = mybir.dt.bfloat16

    nc = bacc.Bacc("TRN2", target_bir_lowering=False, debug=False,
                   num_devices=NC, num_swdge_queues=4)
    xT_d = nc.dram_tensor("xT", [128, HT_COLS], f32, kind="ExternalInput")
    Wk_d = nc.dram_tensor("Wk", [128, L * 128], f32, kind="ExternalInput")
    bias_d = nc.dram_tensor("biasb", [128, L * 128], f32, kind="ExternalInput")
    dinv_d = nc.dram_tensor("dinvc", [128, NBLK], f32, kind="ExternalInput")
    brel_d = nc.dram_tensor("brel", [128, NBLK], f32, kind="ExternalInput")
    iota_d = nc.dram_tensor("iota", [128, 128], f32, kind="ExternalInput")
    iota3_d = nc.dram_tensor("iota3", [128, SMAX * 128], bf16, kind="ExternalInput")
    hw_d = nc.dram_tensor("hw", [128, 1], f32, kind="ExternalInput")
    idx_d = nc.dram_tensor("idx", [128, idx_cols], mybir.dt.int16, kind="ExternalInput")
    dl_d = nc.dram_tensor("dl", [128, dl_cols], bf16, kind="ExternalInput")
    idx0_d = nc.dram_tensor("idx0", [128, 8], mybir.dt.int16, kind="ExternalInput")
    out_d = nc.dram_tensor("out", [1, POOLW], f32, kind="ExternalOutput")

    with tile.TileContext(nc) as tc:
        with (
            tc.tile_pool(name="const", bufs=1) as cp,
            tc.tile_pool(name="hT", bufs=2) as htp,
            tc.tile_pool(name="tev", bufs=3) as tevp,
            tc.tile_pool(name="tloc", bufs=3) as tlp,
            tc.tile_pool(name="m0", bufs=MSG_BUFS) as mp0,
            tc.tile_pool(name="m1", bufs=MSG_BUFS) as mp1,
            tc.tile_pool(name="m2", bufs=MSG_BUFS) as mp2,
            tc.tile_pool(name="m3", bufs=MSG_BUFS) as mp3,
            tc.tile_pool(name="s0", bufs=S_BUFS) as sp0,
            tc.tile_pool(name="s1", bufs=S_BUFS) as sp1,
            tc.tile_pool(name="s2", bufs=S_BUFS) as sp2,
            tc.tile_pool(name="s3", bufs=S_BUFS) as sp3,
            tc.tile_pool(name="ev", bufs=3) as evp,
            tc.tile_pool(name="psA", bufs=3, space="PSUM") as psA,
            tc.tile_pool(name="psB", bufs=3, space="PSUM") as psB,
            tc.tile_pool(name="psH", bufs=1, space="PSUM") as psH,
            tc.tile_pool(name="dram", bufs=1, space="DRAM") as dp,
        ):
            mpools = [mp0, mp1, mp2, mp3]
            spools = [sp0, sp1, sp2, sp3]
            # constants
            Wk = cp.tile([128, L * 128], f32)
            nc.sync.dma_start(Wk[:], Wk_d[:])
            biasb = cp.tile([128, L * 128], f32)
            nc.sync.dma_start(biasb[:], bias_d[:])
            dinvc = cp.tile([128, NBLK], f32)
            nc.sync.dma_start(dinvc[:], dinv_d[:])
            brel = cp.tile([128, NBLK], f32)
            nc.sync.dma_start(brel[:], brel_d[:])
            iota = cp.tile([128, 128], f32)
            nc.sync.dma_start(iota[:], iota_d[:])
            iota3 = cp.tile([128, SMAX * 128], bf16)
            nc.sync.dma_start(iota3[:], iota3_d[:])
            hw = cp.tile([128, 1], f32)
            nc.sync.dma_start(hw[:], hw_d[:])
            idxt = cp.tile([128, idx_cols], mybir.dt.int16)
            nc.sync.dma_start(idxt[:], idx_d[:])
            dlt = cp.tile([128, dl_cols], bf16)
            nc.sync.dma_start(dlt[:], dl_d[:])
            identb = cp.tile([128, 128], bf16)
            make_identity(nc, identb[:])
            if not struct["bias_zero"]:
                ident = cp.tile([128, 128], f32)
                make_identity(nc, ident[:])
            pooledT = cp.tile([128, POOLW], f32)
            nc.vector.memset(pooledT[:], 0.0)
            dinvc2 = cp.tile([128, NBLK], f32)
            nc.vector.tensor_tensor(out=dinvc2[:], in0=dinvc[:], in1=dinvc[:],
                                    op=mybir.AluOpType.mult)
            # warmup gather: front-load the Q7 library reload at t=0
            idx0t = cp.tile([128, 8], mybir.dt.int16)
            nc.sync.dma_start(idx0t[:], idx0_d[:])
            warm = cp.tile([128, 1, 128], f32)
            nc.gpsimd.dma_gather(
                out_ap=warm[:], in_ap=Wk_d[:, 0:128], idxs_ap=idx0t[:],
                num_idxs=128, num_idxs_reg=128, elem_size=128, elem_step=L * 128,
                single_packet=False, queue_num=0)

            hT_dram = [dp.tile([128, HT_COLS], f32, name=f"hTd{i}") for i in range(2)]
            agin = [[dp.tile([SLICES[q], 128], bf16, name=f"agin{l}_{q}")
                     for q in range(NQ)] for l in range(L)]
            agout = [[dp.tile([NC * SLICES[q], 128], bf16, name=f"agout{l}_{q}",
                              addr_space="Shared")
                      for q in range(NQ)] for l in range(L)]

            def split_rows(r0, r1):
                """Split core-local row range [r0, r1) at pool boundaries ->
                (q, pool_row_start, rel_start, n) pieces."""
                out = []
                rr = r0
                while rr < r1:
                    q = next(i for i in range(NQ) if SB[i] <= rr < SB[i + 1])
                    take = min(r1, SB[q + 1]) - rr
                    out.append((q, rr - SB[q], rr - r0, take))
                    rr += take
                return out

            # ---------------- emission helpers ----------------
            # phase A part p of layer l: blocks 14p..14p+13 -> t' -> agin
            def emit_phaseA_part(l, p):
                b0 = p * APART
                nblk = min(APART, NBLK - b0)
                cols = slice(b0 * 128, (b0 + nblk) * 128)
                hTt = htp.tile([128, APART * 128], f32, tag="hT")
                if l == 0:
                    nc.sync.dma_start(hTt[:, 0:nblk * 128], xT_d[:, cols])
                else:
                    nc.sync.dma_start(hTt[:, 0:nblk * 128],
                                      hT_dram[(l + 1) % 2][:, cols])
                for bi in range(nblk):
                    b = b0 + bi
                    w = BW[b]
                    pt = psA.tile([128, 128], f32, tag="psA")
                    nc.tensor.matmul(pt[0:w, :], lhsT=hTt[:, bi * 128:bi * 128 + w],
                                     rhs=Wk[:, l * 128:(l + 1) * 128],
                                     start=True, stop=True)
                    tev = tevp.tile([128, 128], bf16, tag="tev")
                    # layer 0 input is raw x (scale dinv); later layers read the
                    # unscaled relu output r (h = dinv*r), so scale dinv^2
                    # (bias-zero fast path only).
                    dsc = dinvc if (l == 0 or not struct["bias_zero"]) else dinvc2
                    nc.vector.tensor_scalar_mul(tev[0:w, :], pt[0:w, :],
                                                dsc[0:w, b:b + 1])
                    # write rows into agin slices (may straddle a boundary)
                    for q, ps, rel, take in split_rows(b * 128, b * 128 + w):
                        nc.sync.dma_start(agin[l][q][ps:ps + take, :],
                                          tev[rel:rel + take, :])

            def emit_ag(l, q):
                nc.gpsimd.collective_compute(
                    "AllGather", mybir.AluOpType.bypass,
                    ins=[agin[l][q].opt()], outs=[agout[l][q].opt()],
                    replica_groups=[list(range(NC))],
                )

            # phase-A part threshold -> AG piece ready after the part whose
            # rows first cover the pool's upper boundary
            ag_after_part = {}
            for q in range(NQ):
                p = (SB[q + 1] - 1) // (APART * 128)
                ag_after_part.setdefault(p, []).append(q)

            def emit_window(l, q, w):
                wch = win_wch[(q, w)]
                nidx = win_nidx[(q, w)]
                g = mpools[q].tile([128, WCH, 128], bf16, tag=f"msg{q}")
                icol = idx_col_base[(q, w)]
                nc.gpsimd.dma_gather(
                    out_ap=g[:, 0:wch, :],
                    in_ap=agout[l][q][:],
                    idxs_ap=idxt[:, icol:icol + nidx // 16],
                    num_idxs=nidx, num_idxs_reg=nidx, elem_size=128,
                    single_packet=False, queue_num=q)
                st = spools[q].tile([128, SMAX, 128], bf16, tag=f"S{q}")
                ns = len(win_slots[(q, w)])
                dcol = dl_col_base[(q, w)]
                nc.vector.tensor_tensor(
                    out=st[:, 0:ns, :],
                    in0=dlt[:, dcol:dcol + ns].to_broadcast([128, ns, 128]),
                    in1=iota3[:, 0:ns * 128].rearrange("p (s d) -> p s d", s=ns),
                    op=mybir.AluOpType.is_equal)
                return g, st

            def emit_block(l, b, mtiles, stiles):
                w = BW[b]
                refs = blk_refs[b]
                pa = psB.tile([128, 128], f32, tag="agg")
                tlocb = tlp.tile([128, 128], bf16, tag="tloc")
                for q, ps, rel, take in split_rows(b * 128, b * 128 + w):
                    nc.sync.dma_start(tlocb[rel:rel + take, :],
                                      agin[l][q][ps:ps + take, :])
                if l < 2 and struct["bias_zero"]:
                    # transposed aggregation: pa[f, d] (lhsT/rhs swapped).
                    # bias is zero, so relu(dinv*x) = dinv*relu(x): emit the
                    # unscaled relu; dinv^2 is applied at the next phase A.
                    nc.tensor.matmul(pa[:], lhsT=tlocb[0:w, :], rhs=identb[0:w, :],
                                     start=True, stop=(len(refs) == 0))
                    for i, (qq, ww, slot, ch) in enumerate(refs):
                        nc.tensor.matmul(
                            pa[:], lhsT=mtiles[(qq, ww)][:, ch, :],
                            rhs=stiles[(qq, ww)][:, slot, :],
                            start=False, stop=(i == len(refs) - 1))
                    hs3 = evp.tile([128, 128], f32, tag="hs3")
                    nc.vector.tensor_scalar(out=hs3[:], in0=pa[:], scalar1=0.0,
                                            scalar2=None, op0=mybir.AluOpType.max)
                    nc.sync.dma_start(hT_dram[l % 2][:, b * 128:(b + 1) * 128], hs3[:])
                    return
                if l < 2:
                    # general-bias fallback: [d, f] orientation with transpose
                    nc.tensor.matmul(pa[:], lhsT=identb[0:w, :], rhs=tlocb[0:w, :],
                                     start=True, stop=(len(refs) == 0))
                    for i, (qq, ww, slot, ch) in enumerate(refs):
                        nc.tensor.matmul(
                            pa[:], lhsT=stiles[(qq, ww)][:, slot, :],
                            rhs=mtiles[(qq, ww)][:, ch, :],
                            start=False, stop=(i == len(refs) - 1))
                    hsA = evp.tile([128, 128], f32, tag="hs")
                    nc.vector.tensor_scalar_mul(hsA[0:w, :], pa[0:w, :],
                                                dinvc[0:w, b:b + 1])
                    hsB = evp.tile([128, 128], f32, tag="hs2")
                    nc.vector.tensor_tensor(out=hsB[0:w, :], in0=hsA[0:w, :],
                                            in1=biasb[0:w, l * 128:(l + 1) * 128],
                                            op=mybir.AluOpType.add)
                    hsC = evp.tile([128, 128], f32, tag="hs3")
                    nc.scalar.activation(hsC[0:w, :], hsB[0:w, :],
                                         mybir.ActivationFunctionType.Relu)
                    ptr = psA.tile([128, 128], f32, tag="psA")
                    nc.tensor.transpose(ptr[:], hsC[:], ident[:])
                    hTs = evp.tile([128, 128], f32, tag="hTs")
                    nc.vector.tensor_copy(hTs[:], ptr[:])
                    nc.sync.dma_start(hT_dram[l % 2][:, b * 128:(b + 1) * 128], hTs[:])
                    return
                # layer 2: [d, f] orientation for pooling
                nc.tensor.matmul(pa[:], lhsT=identb[0:w, :], rhs=tlocb[0:w, :],
                                 start=True, stop=(len(refs) == 0))
                for i, (qq, ww, slot, ch) in enumerate(refs):
                    nc.tensor.matmul(
                        pa[:], lhsT=stiles[(qq, ww)][:, slot, :],
                        rhs=mtiles[(qq, ww)][:, ch, :],
                        start=False, stop=(i == len(refs) - 1))
                hs = evp.tile([128, 128], f32, tag="hs")
                nc.vector.tensor_scalar_mul(hs[0:w, :], pa[0:w, :], dinvc[0:w, b:b + 1])
                hs2 = evp.tile([128, 128], f32, tag="hs2")
                nc.vector.tensor_tensor(out=hs2[0:w, :], in0=hs[0:w, :],
                                        in1=biasb[0:w, l * 128:(l + 1) * 128],
                                        op=mybir.AluOpType.add)
                hs3 = evp.tile([128, 128], f32, tag="hs3")
                nc.scalar.activation(hs3[0:w, :], hs2[0:w, :],
                                     mybir.ActivationFunctionType.Relu)
                spool_t = evp.tile([128, 128], f32, tag="spool")
                nc.vector.tensor_tensor(
                    out=spool_t[:], in0=brel[:, b:b + 1].to_broadcast([128, 128]),
                    in1=iota[:], op=mybir.AluOpType.is_equal)
                pp = psA.tile([128, 128], f32, tag="psA")
                nc.tensor.matmul(pp[:], lhsT=hs3[:], rhs=spool_t[:],
                                 start=True, stop=True)
                wsb = ws_blk[b]
                nc.vector.tensor_tensor(
                    out=pooledT[:, wsb:wsb + 128], in0=pooledT[:, wsb:wsb + 128],
                    in1=pp[:], op=mybir.AluOpType.add)

            # ---------------- pipelined driver ----------------
            # warm-start stagger: AG pieces land sequentially at layer-0 start,
            # so delay each pool's first windows; lockstep after (monotone per
            # pool so `prog` stays valid).
            items = [(max(w, STAG * q), q, w)
                     for q in range(NQ) for w in range(NW[q])]
            items.sort()
            worder = [(q, w) for _, q, w in items]

            # phase A layer 0 fully up front (+ AGs at thresholds)
            for p in range(7):
                emit_phaseA_part(0, p)
                for q in ag_after_part.get(p, []):
                    emit_ag(0, q)

            for l in range(L):
                mtiles, stiles = {}, {}
                prog = [0] * NQ              # windows emitted per pool
                state = {"emitted": 0, "partsA": 0}

                def block_ready(b):
                    return all(prog[q] > blk_lastw[b][q] for q in range(NQ))

                def drain_ready(check=True):
                    while state["emitted"] < NBLK and (
                            not check or block_ready(state["emitted"])):
                        emit_block(l, state["emitted"], mtiles, stiles)
                        state["emitted"] += 1
                        if l < 2:
                            while (state["partsA"] < 7 and state["emitted"]
                                   >= APART * (state["partsA"] + 1)):
                                emit_phaseA_part(l + 1, state["partsA"])
                                for q in ag_after_part.get(state["partsA"], []):
                                    emit_ag(l + 1, q)
                                state["partsA"] += 1

                for (q, w) in worder:
                    g, st = emit_window(l, q, w)
                    mtiles[(q, w)] = g
                    stiles[(q, w)] = st
                    prog[q] = w + 1
                    drain_ready()
                drain_ready(check=False)

            # ---------- head: partial logits ----------
            ph = psH.tile([128, POOLW], f32)
            nc.tensor.matmul(ph[0:1, :], lhsT=hw[:, 0:1], rhs=pooledT[:],
                             start=True, stop=True)
            outsb = cp.tile([1, POOLW], f32)
            nc.vector.tensor_copy(outsb[:], ph[0:1, :])
            nc.sync.dma_start(out_d[:], outsb[:])
    nc.compile()
    return nc


# ---------------------------------------------------------------------------
# PJRT compile-once runner (inlined; mirrors concourse.bass2jax.run_bass_via_pjrt)
# ---------------------------------------------------------------------------
class _Runner:
    def __init__(self, nc, n_cores):
        import jax
        import numpy as np
        from jax.sharding import Mesh, PartitionSpec
        from jax.experimental.shard_map import shard_map
        import concourse.mybir as mybir
        from concourse import bass2jax
        from concourse.bass2jax import _bass_exec_p, partition_id_tensor

        bass2jax.install_neuronx_cc_hook()
        self.jax = jax
        self.n_cores = n_cores
        partition_name = nc.partition_id_tensor.name if nc.partition_id_tensor else None
        in_names, out_names, out_avals, zero_outs = [], [], [], []
        for alloc in nc.m.functions[0].allocations:
            if not isinstance(alloc, mybir.MemoryLocationSet):
                continue
            name = alloc.memorylocations[0].name
            if alloc.kind == "ExternalInput":
                if name != partition_name:
                    in_names.append(name)
            elif alloc.kind == "ExternalOutput":
                out_names.append(name)
                out_avals.append(jax.core.ShapedArray(tuple(alloc.tensor_shape),
                                                      mybir.dt.np(alloc.dtype)))
                zero_outs.append(np.zeros(tuple(alloc.tensor_shape),
                                          mybir.dt.np(alloc.dtype)))
        self.in_names, self.out_names = in_names, out_names
        self.out_avals, self.zero_outs = out_avals, zero_outs
        n_params, n_outs = len(in_names), len(out_avals)
        all_in = list(in_names) + list(out_names)
        if partition_name is not None:
            all_in.append(partition_name)

        def _body(*args):
            operands = list(args)
            if partition_name is not None:
                operands.append(partition_id_tensor())
            return tuple(_bass_exec_p.bind(
                *operands, out_avals=tuple(out_avals), in_names=tuple(all_in),
                out_names=tuple(out_names), lowering_input_output_aliases=(),
                sim_require_finite=False, sim_require_nnan=False, nc=nc))

        devices = jax.devices()[:n_cores]
        self.mesh = Mesh(np.asarray(devices), ("core",))
        in_specs = (PartitionSpec("core"),) * (n_params + n_outs)
        out_specs = (PartitionSpec("core"),) * n_outs
        self.sharded = jax.jit(
            shard_map(_body, mesh=self.mesh, in_specs=in_specs,
                      out_specs=out_specs, check_rep=False),
            donate_argnums=tuple(range(n_params, n_params + n_outs)),
            keep_unused=True)

    def run(self, in_maps):
        import numpy as np
        from jax.sharding import NamedSharding, PartitionSpec
        sharding = NamedSharding(self.mesh, PartitionSpec("core"))
        concat = [self.jax.device_put(
            np.concatenate([np.asarray(in_maps[c][n]) for c in range(self.n_cores)], axis=0),
            sharding) for n in self.in_names]
        zeros = [self.jax.device_put(
            np.zeros((self.n_cores * z.shape[0], *z.shape[1:]), z.dtype), sharding)
            for z in self.zero_outs]
        outs = self.sharded(*concat, *zeros)
        self.jax.block_until_ready(outs)
        return [
            {n: np.asarray(outs[i]).reshape(self.n_cores, *self.out_avals[i].shape)[c]
             for i, n in enumerate(self.out_names)}
            for c in range(self.n_cores)
        ]


_CACHE = {}


def kernel(x, edge_index, batch, Ws, bs, head_w, head_b):
    import hashlib
    ins_per_core, struct = _prep(x, edge_index, batch, Ws, bs, head_w, head_b)
    h = hashlib.sha1()
    h.update(np.ascontiguousarray(edge_index).tobytes())
    h.update(np.ascontiguousarray(batch).tobytes())
    key = h.hexdigest()
    if key not in _CACHE:
        nc = _build(struct)
        _CACHE[key] = _Runner(nc, NC)
        _CACHE["gcn"] = _CACHE[key]
    runner = _CACHE[key]
    results = runner.run(ins_per_core)
    out = np.zeros(G, np.float64)
    for c in range(NC):
        part = results[c]["out"].reshape(-1)
        g0 = int(struct["pooled_base"][c])
        w = min(POOLW, G - g0)
        out[g0:g0 + w] += part[:w]
    out += struct["head_b"]
    return out.astype(np.float32)



# revision 1
# speedup vs baseline: 1.0053x; 1.0053x over previous
"""SimpleGCN (3-layer GCNConv + global_add_pool + linear head) on 8 Trainium2 cores.

Strategy (self-contained; shapes hardcoded for the nn_SimpleGCN problem):
 - Nodes sharded contiguously across 8 cores by dst (12500 each).
 - Per layer, per core: t' = (h @ W) * dinv[node] for the local shard (PE),
   written bf16 (256B rows); broadcast via FOUR pipelined AllGathers, one per
   "pool" (pool q = concat over cores of their q-th 3125-row sub-slice), so
   gathers for pool q start as soon as AG_q lands.
 - Message aggregation: edges (self-loops EXCLUDED) sorted per (dst-block,
   src-pool), densely packed into 128-edge chunks (chunks may span two dst
   blocks; each (chunk, block) pair gets its own one-hot S slot built with
   is_equal), dma_gather pulls t'[src] rows (int16 pool-local indices),
   PE matmul-accumulates per-128-dst-block segment sums in PSUM.
 - Self-loop contribution added per block via identity-matmul of the local
   t' block (DMA'd back from agin) into the same PSUM group.
 - h' = relu(dinv*psum + bias); transposed (PE) back to feat-major for next
   layer. Emission order is software-pipelined: phase A / AG of layer l+1 are
   emitted as soon as the blocks they need are emitted in layer l, so the
   GpSimd gather-descriptor stream (the bottleneck engine) never stalls at
   layer boundaries.
 - Layer 3: per-block pooling matmul into a per-core local-graph window;
   head matmul gives per-core partial logits; host sums partials + head_b.
"""
import math
import numpy as np

N_NODES = 100000
N_EDGES = 1600000
D = 128
L = 3
G = 512
NC = 8
SH = N_NODES // NC            # 12500 nodes per core
NBLK = math.ceil(SH / 128)    # 98 blocks (97 full + one of 84)
BW = [128] * (NBLK - 1) + [SH - 128 * (NBLK - 1)]
NQ = 4                        # pools (AllGather pieces)
# every pool must satisfy 8*rows <= 32767 (int16 gather indices).
SLICES = [3125, 3125, 3125, 3125]
SB = [0, 3125, 6250, 9375, 12500]   # per-core pool row boundaries
WCH = 14                      # msg chunks per gather window
MSG_BUFS = 5
S_BUFS = 2
HT_COLS = NBLK * 128          # 12544 (padded node cols)
POOLW = 256                   # per-core local pooled window
STAG = 3                      # warm-start stagger (first windows only)
APART = 14                    # phase-A blocks per part (7 parts)


def _prep(x, edge_index, batch, Ws, bs, head_w, head_b):
    x = np.asarray(x, np.float32)
    ei = np.asarray(edge_index, np.int64)
    batch = np.asarray(batch, np.int64)
    Ws = np.asarray(Ws, np.float32)
    bs = np.asarray(bs, np.float32)
    head_w = np.asarray(head_w, np.float32)

    # degree includes self-loops (as in reference)
    deg = np.bincount(ei[1], minlength=N_NODES).astype(np.float32) + 1.0
    dinv = (1.0 / np.sqrt(deg)).astype(np.float32)

    src_all, dst_all = ei[0], ei[1]

    # ---- per-core edge streams by (dst block, src pool); no self-loops ----
    core = dst_all // SH
    sb_arr = np.asarray(SB[1:], np.int64)
    per_core = []                    # (src_sorted, dloc_sorted, group starts)
    counts = np.zeros((NC, NBLK * NQ), np.int64)
    for c in range(NC):
        m = core == c
        s_c = src_all[m]
        dloc = dst_all[m] - c * SH
        b = dloc >> 7
        q = np.searchsorted(sb_arr, s_c % SH, side="right")   # src pool
        key = b * NQ + q
        order = np.argsort(key, kind="stable")
        counts[c] = np.bincount(key, minlength=NBLK * NQ)
        per_core.append((s_c[order], dloc[order], np.cumsum(counts[c]) - counts[c]))

    cmax = counts.max(axis=0).reshape(NBLK, NQ)     # common group sizes

    # ---- dense slot layout per pool (shared across cores) ----
    # pool q stream: groups (b=0..97, q) concatenated, then chunked at 128.
    grp_base = np.zeros((NBLK, NQ), np.int64)       # slot offset within pool
    pool_slots = np.zeros(NQ, np.int64)
    for q in range(NQ):
        run = 0
        for b in range(NBLK):
            grp_base[b, q] = run
            run += int(cmax[b, q])
        pool_slots[q] = run
    NCHUNK = [int(-(-pool_slots[q] // 128)) for q in range(NQ)]
    NW = [int(-(-NCHUNK[q] // WCH)) for q in range(NQ)]

    # per (q, chunk): list of dst blocks present -> S slots
    # slot table: per (q, w): list of (chunk_in_window, bvar)
    win_slots = {}                                  # (q, w) -> list
    blk_refs = [[] for _ in range(NBLK)]            # b -> (q, w, slot, chunk)
    blk_lastw = np.zeros((NBLK, NQ), np.int64)      # last window of b in q
    for q in range(NQ):
        for b in range(NBLK):
            n = int(cmax[b, q])
            if n == 0:
                continue
            st = int(grp_base[b, q])
            c0, c1 = st // 128, (st + n - 1) // 128
            for ch in range(c0, c1 + 1):
                w = ch // WCH
                key = (q, w)
                slots = win_slots.setdefault(key, [])
                slot = len(slots)
                slots.append((ch % WCH, b))
                blk_refs[b].append((q, w, slot, ch % WCH))
            blk_lastw[b, q] = c1 // WCH
    SMAX = max(len(v) for v in win_slots.values())

    # window metadata (structural, same for all cores)
    win_wch = {}
    win_nidx = {}
    for q in range(NQ):
        for w in range(NW[q]):
            lo, hi = w * WCH, min((w + 1) * WCH, NCHUNK[q])
            win_wch[(q, w)] = hi - lo
            win_nidx[(q, w)] = min((hi - lo) * 128, 10 ** 9)
    # last window of pool q covers chunks up to NCHUNK[q]; all its slots are
    # gathered (pad idx 0) so msg rows are always defined.

    # column layouts
    # idx: per (q, w) wrapped [16, nidx/16] replicated x8 -> [128, nidx/16]
    idx_cols_per = {k: win_nidx[k] // 16 for k in win_nidx}
    # dl: per (q, w): SMAX slot columns
    order_keys = [(q, w) for q in range(NQ) for w in range(NW[q])]
    idx_col_base = {}
    run = 0
    for k in order_keys:
        idx_col_base[k] = run
        run += idx_cols_per[k]
    idx_cols = run
    dl_col_base = {k: i * SMAX for i, k in enumerate(order_keys)}
    dl_cols = len(order_keys) * SMAX

    # block ready rule: window-progress vector (per q) must reach blk_lastw[b]
    # pooled windows
    ws_blk = [max(0, int(b * 128 * G / N_NODES) - 32) for b in range(NBLK)]

    ins_per_core = []
    pooled_base = np.zeros(NC, np.int64)
    for c in range(NC):
        s_c, dloc_c, starts = per_core[c]
        idx_flat = {}                               # (q,) -> pool slot idx array
        dlv = np.full((dl_cols, 128), -1.0, np.float32)   # [slotcol, row]
        ixv = {q: np.zeros(int(-(-pool_slots[q] // 128)) * 128, np.int64)
               for q in range(NQ)}
        dloc_slot = {q: np.full(int(-(-pool_slots[q] // 128)) * 128, -(10 ** 6),
                                np.int64) for q in range(NQ)}
        for q in range(NQ):
            for b in range(NBLK):
                n = int(counts[c][b * NQ + q])
                if n == 0:
                    continue
                st = int(starts[b * NQ + q])
                base = int(grp_base[b, q])
                s_grp = s_c[st:st + n]
                pos = (s_grp // SH) * SLICES[q] + (s_grp % SH) - SB[q]
                ixv[q][base:base + n] = pos
                dloc_slot[q][base:base + n] = dloc_c[st:st + n]
        # dl slot columns
        for (q, w), slots in win_slots.items():
            for slot, (ch, bvar) in enumerate(slots):
                ch_g = w * WCH + ch
                rows = dloc_slot[q][ch_g * 128:(ch_g + 1) * 128]
                col = dl_col_base[(q, w)] + slot
                v = rows.astype(np.float64) - 128.0 * bvar
                v[rows < 0] = -1.0
                dlv[col, :len(rows)] = v.astype(np.float32)
        # idx wrapped layout
        idx_sb = np.zeros((128, idx_cols), np.int64)
        for (q, w) in order_keys:
            nidx = win_nidx[(q, w)]
            base_slot = w * WCH * 128
            vals = ixv[q][base_slot:base_slot + nidx]
            arr = vals.reshape(nidx // 16, 16)       # [j//16, j%16]
            wrapped = arr.T                          # [16, nidx/16]
            rep = np.tile(wrapped, (8, 1))           # [128, nidx/16]
            cb = idx_col_base[(q, w)]
            idx_sb[:, cb:cb + nidx // 16] = rep

        xT = np.zeros((128, HT_COLS), np.float32)
        xT[:, :SH] = x[c * SH:(c + 1) * SH].T
        dinv_c = np.ones((128, NBLK), np.float32)
        dv = dinv[c * SH:(c + 1) * SH]
        for b in range(NBLK):
            dinv_c[:BW[b], b] = dv[b * 128:b * 128 + BW[b]]
        bl = batch[c * SH:(c + 1) * SH]
        g0 = int(bl[0])
        pooled_base[c] = g0
        brel = np.full((128, NBLK), -1.0, np.float32)
        for b in range(NBLK):
            rel = (bl[b * 128:b * 128 + BW[b]] - g0 - ws_blk[b]).astype(np.int64)
            assert rel.min() >= 0 and rel.max() < 128, (c, b, rel.min(), rel.max())
            brel[:BW[b], b] = rel.astype(np.float32)
        iota = np.broadcast_to(np.arange(128, dtype=np.float32), (128, 128)).copy()
        iota3 = np.tile(np.arange(128, dtype=np.float32), (128, SMAX)).copy()
        Wk = np.ascontiguousarray(Ws.transpose(1, 0, 2).reshape(128, L * 128))
        bias_b = np.ascontiguousarray(
            np.broadcast_to(bs[:, None, :], (L, 128, 128)).transpose(1, 0, 2).reshape(128, L * 128))
        import ml_dtypes
        ins_per_core.append({
            "xT": xT, "Wk": Wk, "biasb": bias_b, "dinvc": dinv_c, "brel": brel,
            "iota": iota,
            "iota3": iota3.astype(ml_dtypes.bfloat16),
            "hw": head_w.reshape(128, 1).astype(np.float32),
            "idx": idx_sb.astype(np.int16),
            "dl": np.ascontiguousarray(dlv.T).astype(ml_dtypes.bfloat16),
            "idx0": np.zeros((128, 8), np.int16),
        })
    struct = {
        "bias_zero": bool(np.all(bs == 0.0)),
        "NW": NW, "SMAX": SMAX, "idx_cols": idx_cols, "dl_cols": dl_cols,
        "idx_col_base": idx_col_base, "dl_col_base": dl_col_base,
        "win_slots": win_slots, "win_wch": win_wch, "win_nidx": win_nidx,
        "blk_refs": blk_refs, "blk_lastw": blk_lastw,
        "ws_blk": ws_blk, "pooled_base": pooled_base,
        "head_b": float(np.asarray(head_b).reshape(-1)[0]),
    }
    return ins_per_core, struct


def _build(struct):
    import concourse.bass as bass
    import concourse.bacc as bacc
    import concourse.mybir as mybir
    import concourse.tile as tile
    from concourse.masks import make_identity

    NW = struct["NW"]
    SMAX = struct["SMAX"]
    idx_cols = struct["idx_cols"]
    dl_cols = struct["dl_cols"]
    idx_col_base = struct["idx_col_base"]
    dl_col_base = struct["dl_col_base"]
    win_slots = struct["win_slots"]
    win_wch = struct["win_wch"]
    win_nidx = struct["win_nidx"]
    blk_refs = struct["blk_refs"]
    blk_lastw = struct["blk_lastw"]
    ws_blk = struct["ws_blk"]
    f32 = mybir.dt.float32
    bf16 = mybir.dt.bfloat16

    nc = bacc.Bacc("TRN2", target_bir_lowering=False, debug=False,
                   num_devices=NC, num_swdge_queues=4)
    xT_d = nc.dram_tensor("xT", [128, HT_COLS], f32, kind="ExternalInput")
    Wk_d = nc.dram_tensor("Wk", [128, L * 128], f32, kind="ExternalInput")
    bias_d = nc.dram_tensor("biasb", [128, L * 128], f32, kind="ExternalInput")
    dinv_d = nc.dram_tensor("dinvc", [128, NBLK], f32, kind="ExternalInput")
    brel_d = nc.dram_tensor("brel", [128, NBLK], f32, kind="ExternalInput")
    iota_d = nc.dram_tensor("iota", [128, 128], f32, kind="ExternalInput")
    iota3_d = nc.dram_tensor("iota3", [128, SMAX * 128], bf16, kind="ExternalInput")
    hw_d = nc.dram_tensor("hw", [128, 1], f32, kind="ExternalInput")
    idx_d = nc.dram_tensor("idx", [128, idx_cols], mybir.dt.int16, kind="ExternalInput")
    dl_d = nc.dram_tensor("dl", [128, dl_cols], bf16, kind="ExternalInput")
    idx0_d = nc.dram_tensor("idx0", [128, 8], mybir.dt.int16, kind="ExternalInput")
    out_d = nc.dram_tensor("out", [1, POOLW], f32, kind="ExternalOutput")

    with tile.TileContext(nc) as tc:
        with (
            tc.tile_pool(name="const", bufs=1) as cp,
            tc.tile_pool(name="hT", bufs=2) as htp,
            tc.tile_pool(name="tev", bufs=3) as tevp,
            tc.tile_pool(name="tloc", bufs=3) as tlp,
            tc.tile_pool(name="m0", bufs=MSG_BUFS) as mp0,
            tc.tile_pool(name="m1", bufs=MSG_BUFS) as mp1,
            tc.tile_pool(name="m2", bufs=MSG_BUFS) as mp2,
            tc.tile_pool(name="m3", bufs=MSG_BUFS) as mp3,
            tc.tile_pool(name="s0", bufs=S_BUFS) as sp0,
            tc.tile_pool(name="s1", bufs=S_BUFS) as sp1,
            tc.tile_pool(name="s2", bufs=S_BUFS) as sp2,
            tc.tile_pool(name="s3", bufs=S_BUFS) as sp3,
            tc.tile_pool(name="ev", bufs=3) as evp,
            tc.tile_pool(name="psA", bufs=3, space="PSUM") as psA,
            tc.tile_pool(name="psB", bufs=3, space="PSUM") as psB,
            tc.tile_pool(name="psH", bufs=1, space="PSUM") as psH,
            tc.tile_pool(name="dram", bufs=1, space="DRAM") as dp,
        ):
            mpools = [mp0, mp1, mp2, mp3]
            spools = [sp0, sp1, sp2, sp3]
            # constants
            Wk = cp.tile([128, L * 128], f32)
            nc.sync.dma_start(Wk[:], Wk_d[:])
            biasb = cp.tile([128, L * 128], f32)
            nc.sync.dma_start(biasb[:], bias_d[:])
            dinvc = cp.tile([128, NBLK], f32)
            nc.sync.dma_start(dinvc[:], dinv_d[:])
            brel = cp.tile([128, NBLK], f32)
            nc.sync.dma_start(brel[:], brel_d[:])
            iota = cp.tile([128, 128], f32)
            nc.sync.dma_start(iota[:], iota_d[:])
            iota3 = cp.tile([128, SMAX * 128], bf16)
            nc.sync.dma_start(iota3[:], iota3_d[:])
            hw = cp.tile([128, 1], f32)
            nc.sync.dma_start(hw[:], hw_d[:])
            idxt = cp.tile([128, idx_cols], mybir.dt.int16)
            nc.sync.dma_start(idxt[:], idx_d[:])
            dlt = cp.tile([128, dl_cols], bf16)
            nc.sync.dma_start(dlt[:], dl_d[:])
            identb = cp.tile([128, 128], bf16)
            make_identity(nc, identb[:])
            if not struct["bias_zero"]:
                ident = cp.tile([128, 128], f32)
                make_identity(nc, ident[:])
            pooledT = cp.tile([128, POOLW], f32)
            nc.vector.memset(pooledT[:], 0.0)
            dinvc2 = cp.tile([128, NBLK], f32)
            nc.vector.tensor_tensor(out=dinvc2[:], in0=dinvc[:], in1=dinvc[:],
                                    op=mybir.AluOpType.mult)
            # warmup gather: front-load the Q7 library reload at t=0
            idx0t = cp.tile([128, 8], mybir.dt.int16)
            nc.sync.dma_start(idx0t[:], idx0_d[:])
            warm = cp.tile([128, 1, 128], f32)
            nc.gpsimd.dma_gather(
                out_ap=warm[:], in_ap=Wk_d[:, 0:128], idxs_ap=idx0t[:],
                num_idxs=128, num_idxs_reg=128, elem_size=128, elem_step=L * 128,
                single_packet=False, queue_num=0)

            hT_dram = [dp.tile([128, HT_COLS], f32, name=f"hTd{i}") for i in range(2)]
            agin = [[dp.tile([SLICES[q], 128], bf16, name=f"agin{l}_{q}")
                     for q in range(NQ)] for l in range(L)]
            agout = [[dp.tile([NC * SLICES[q], 128], bf16, name=f"agout{l}_{q}",
                              addr_space="Shared")
                      for q in range(NQ)] for l in range(L)]

            def split_rows(r0, r1):
                """Split core-local row range [r0, r1) at pool boundaries ->
                (q, pool_row_start, rel_start, n) pieces."""
                out = []
                rr = r0
                while rr < r1:
                    q = next(i for i in range(NQ) if SB[i] <= rr < SB[i + 1])
                    take = min(r1, SB[q + 1]) - rr
                    out.append((q, rr - SB[q], rr - r0, take))
                    rr += take
                return out

            # ---------------- emission helpers ----------------
            # phase A part p of layer l: blocks 14p..14p+13 -> t' -> agin
            def emit_phaseA_part(l, p):
                b0 = p * APART
                nblk = min(APART, NBLK - b0)
                cols = slice(b0 * 128, (b0 + nblk) * 128)
                hTt = htp.tile([128, APART * 128], f32, tag="hT")
                if l == 0:
                    nc.sync.dma_start(hTt[:, 0:nblk * 128], xT_d[:, cols])
                else:
                    nc.sync.dma_start(hTt[:, 0:nblk * 128],
                                      hT_dram[(l + 1) % 2][:, cols])
                for bi in range(nblk):
                    b = b0 + bi
                    w = BW[b]
                    pt = psA.tile([128, 128], f32, tag="psA")
                    nc.tensor.matmul(pt[0:w, :], lhsT=hTt[:, bi * 128:bi * 128 + w],
                                     rhs=Wk[:, l * 128:(l + 1) * 128],
                                     start=True, stop=True)
                    tev = tevp.tile([128, 128], bf16, tag="tev")
                    # layer 0 input is raw x (scale dinv); later layers read the
                    # unscaled relu output r (h = dinv*r), so scale dinv^2
                    # (bias-zero fast path only).
                    dsc = dinvc if (l == 0 or not struct["bias_zero"]) else dinvc2
                    nc.vector.tensor_scalar_mul(tev[0:w, :], pt[0:w, :],
                                                dsc[0:w, b:b + 1])
                    # write rows into agin slices (may straddle a boundary)
                    for q, ps, rel, take in split_rows(b * 128, b * 128 + w):
                        nc.sync.dma_start(agin[l][q][ps:ps + take, :],
                                          tev[rel:rel + take, :])

            def emit_ag(l, q):
                nc.gpsimd.collective_compute(
                    "AllGather", mybir.AluOpType.bypass,
                    ins=[agin[l][q].opt()], outs=[agout[l][q].opt()],
                    replica_groups=[list(range(NC))],
                )

            # phase-A part threshold -> AG piece ready after the part whose
            # rows first cover the pool's upper boundary
            ag_after_part = {}
            for q in range(NQ):
                p = (SB[q + 1] - 1) // (APART * 128)
                ag_after_part.setdefault(p, []).append(q)

            def emit_window(l, q, w):
                wch = win_wch[(q, w)]
                nidx = win_nidx[(q, w)]
                g = mpools[q].tile([128, WCH, 128], bf16, tag=f"msg{q}")
                icol = idx_col_base[(q, w)]
                nc.gpsimd.dma_gather(
                    out_ap=g[:, 0:wch, :],
                    in_ap=agout[l][q][:],
                    idxs_ap=idxt[:, icol:icol + nidx // 16],
                    num_idxs=nidx, num_idxs_reg=nidx, elem_size=128,
                    single_packet=False, queue_num=q)
                st = spools[q].tile([128, SMAX, 128], bf16, tag=f"S{q}")
                ns = len(win_slots[(q, w)])
                dcol = dl_col_base[(q, w)]
                nc.vector.tensor_tensor(
                    out=st[:, 0:ns, :],
                    in0=dlt[:, dcol:dcol + ns].to_broadcast([128, ns, 128]),
                    in1=iota3[:, 0:ns * 128].rearrange("p (s d) -> p s d", s=ns),
                    op=mybir.AluOpType.is_equal)
                return g, st

            def emit_block(l, b, mtiles, stiles):
                w = BW[b]
                refs = blk_refs[b]
                pa = psB.tile([128, 128], f32, tag="agg")
                tlocb = tlp.tile([128, 128], bf16, tag="tloc")
                for q, ps, rel, take in split_rows(b * 128, b * 128 + w):
                    nc.sync.dma_start(tlocb[rel:rel + take, :],
                                      agin[l][q][ps:ps + take, :])
                if l < 2 and struct["bias_zero"]:
                    # transposed aggregation: pa[f, d] (lhsT/rhs swapped).
                    # bias is zero, so relu(dinv*x) = dinv*relu(x): emit the
                    # unscaled relu; dinv^2 is applied at the next phase A.
                    nc.tensor.matmul(pa[:], lhsT=tlocb[0:w, :], rhs=identb[0:w, :],
                                     start=True, stop=(len(refs) == 0))
                    for i, (qq, ww, slot, ch) in enumerate(refs):
                        nc.tensor.matmul(
                            pa[:], lhsT=mtiles[(qq, ww)][:, ch, :],
                            rhs=stiles[(qq, ww)][:, slot, :],
                            start=False, stop=(i == len(refs) - 1))
                    hs3 = evp.tile([128, 128], f32, tag="hs3")
                    nc.vector.tensor_scalar(out=hs3[:], in0=pa[:], scalar1=0.0,
                                            scalar2=None, op0=mybir.AluOpType.max)
                    nc.sync.dma_start(hT_dram[l % 2][:, b * 128:(b + 1) * 128], hs3[:])
                    return
                if l < 2:
                    # general-bias fallback: [d, f] orientation with transpose
                    nc.tensor.matmul(pa[:], lhsT=identb[0:w, :], rhs=tlocb[0:w, :],
                                     start=True, stop=(len(refs) == 0))
                    for i, (qq, ww, slot, ch) in enumerate(refs):
                        nc.tensor.matmul(
                            pa[:], lhsT=stiles[(qq, ww)][:, slot, :],
                            rhs=mtiles[(qq, ww)][:, ch, :],
                            start=False, stop=(i == len(refs) - 1))
                    hsA = evp.tile([128, 128], f32, tag="hs")
                    nc.vector.tensor_scalar_mul(hsA[0:w, :], pa[0:w, :],
                                                dinvc[0:w, b:b + 1])
                    hsB = evp.tile([128, 128], f32, tag="hs2")
                    nc.vector.tensor_tensor(out=hsB[0:w, :], in0=hsA[0:w, :],
                                            in1=biasb[0:w, l * 128:(l + 1) * 128],
                                            op=mybir.AluOpType.add)
                    hsC = evp.tile([128, 128], f32, tag="hs3")
                    nc.scalar.activation(hsC[0:w, :], hsB[0:w, :],
                                         mybir.ActivationFunctionType.Relu)
                    ptr = psA.tile([128, 128], f32, tag="psA")
                    nc.tensor.transpose(ptr[:], hsC[:], ident[:])
                    hTs = evp.tile([128, 128], f32, tag="hTs")
                    nc.vector.tensor_copy(hTs[:], ptr[:])
                    nc.sync.dma_start(hT_dram[l % 2][:, b * 128:(b + 1) * 128], hTs[:])
                    return
                # layer 2: [d, f] orientation for pooling
                nc.tensor.matmul(pa[:], lhsT=identb[0:w, :], rhs=tlocb[0:w, :],
                                 start=True, stop=(len(refs) == 0))
                for i, (qq, ww, slot, ch) in enumerate(refs):
                    nc.tensor.matmul(
                        pa[:], lhsT=stiles[(qq, ww)][:, slot, :],
                        rhs=mtiles[(qq, ww)][:, ch, :],
                        start=False, stop=(i == len(refs) - 1))
                hs = evp.tile([128, 128], f32, tag="hs")
                nc.vector.tensor_scalar_mul(hs[0:w, :], pa[0:w, :], dinvc[0:w, b:b + 1])
                hs2 = evp.tile([128, 128], f32, tag="hs2")
                nc.vector.tensor_tensor(out=hs2[0:w, :], in0=hs[0:w, :],
                                        in1=biasb[0:w, l * 128:(l + 1) * 128],
                                        op=mybir.AluOpType.add)
                hs3 = evp.tile([128, 128], f32, tag="hs3")
                nc.scalar.activation(hs3[0:w, :], hs2[0:w, :],
                                     mybir.ActivationFunctionType.Relu)
                spool_t = evp.tile([128, 128], f32, tag="spool")
                nc.vector.tensor_tensor(
                    out=spool_t[:], in0=brel[:, b:b + 1].to_broadcast([128, 128]),
                    in1=iota[:], op=mybir.AluOpType.is_equal)
                pp = psA.tile([128, 128], f32, tag="psA")
                nc.tensor.matmul(pp[:], lhsT=hs3[:], rhs=spool_t[:],
                                 start=True, stop=True)
                wsb = ws_blk[b]
                nc.vector.tensor_tensor(
                    out=pooledT[:, wsb:wsb + 128], in0=pooledT[:, wsb:wsb + 128],
                    in1=pp[:], op=mybir.AluOpType.add)

            # ---------------- pipelined driver ----------------
            # warm-start stagger: AG pieces land sequentially at layer-0 start,
            # so delay each pool's first windows; lockstep after (monotone per
            # pool so `prog` stays valid).
            items = [(max(w, STAG * q), q, w)
                     for q in range(NQ) for w in range(NW[q])]
            items.sort()
            worder = [(q, w) for _, q, w in items]

            # phase A layer 0 fully up front (+ AGs at thresholds)
            for p in range(7):
                emit_phaseA_part(0, p)
                for q in ag_after_part.get(p, []):
                    emit_ag(0, q)

            for l in range(L):
                mtiles, stiles = {}, {}
                prog = [0] * NQ              # windows emitted per pool
                state = {"emitted": 0, "partsA": 0}

                def block_ready(b):
                    return all(prog[q] > blk_lastw[b][q] for q in range(NQ))

                def drain_ready(check=True):
                    while state["emitted"] < NBLK and (
                            not check or block_ready(state["emitted"])):
                        emit_block(l, state["emitted"], mtiles, stiles)
                        state["emitted"] += 1
                        if l < 2:
                            while (state["partsA"] < 7 and state["emitted"]
                                   >= APART * (state["partsA"] + 1)):
                                emit_phaseA_part(l + 1, state["partsA"])
                                for q in ag_after_part.get(state["partsA"], []):
                                    emit_ag(l + 1, q)
                                state["partsA"] += 1

                for (q, w) in worder:
                    g, st = emit_window(l, q, w)
                    mtiles[(q, w)] = g
                    stiles[(q, w)] = st
                    prog[q] = w + 1
                    drain_ready()
                drain_ready(check=False)

            # ---------- head: partial logits ----------
            ph = psH.tile([128, POOLW], f32)
            nc.tensor.matmul(ph[0:1, :], lhsT=hw[:, 0:1], rhs=pooledT[:],
                             start=True, stop=True)
            outsb = cp.tile([1, POOLW], f32)
            nc.vector.tensor_copy(outsb[:], ph[0:1, :])
            nc.sync.dma_start(out_d[:], outsb[:])
    nc.compile()
    return nc


# ---------------------------------------------------------------------------
# PJRT compile-once runner (inlined; mirrors concourse.bass2jax.run_bass_via_pjrt)
# ---------------------------------------------------------------------------
class _Runner:
    def __init__(self, nc, n_cores):
        import jax
        import numpy as np
        from jax.sharding import Mesh, PartitionSpec
        from jax.experimental.shard_map import shard_map
        import concourse.mybir as mybir
        from concourse import bass2jax
        from concourse.bass2jax import _bass_exec_p, partition_id_tensor

        bass2jax.install_neuronx_cc_hook()
        self.jax = jax
        self.n_cores = n_cores
        partition_name = nc.partition_id_tensor.name if nc.partition_id_tensor else None
        in_names, out_names, out_avals, zero_outs = [], [], [], []
        for alloc in nc.m.functions[0].allocations:
            if not isinstance(alloc, mybir.MemoryLocationSet):
                continue
            name = alloc.memorylocations[0].name
            if alloc.kind == "ExternalInput":
                if name != partition_name:
                    in_names.append(name)
            elif alloc.kind == "ExternalOutput":
                out_names.append(name)
                out_avals.append(jax.core.ShapedArray(tuple(alloc.tensor_shape),
                                                      mybir.dt.np(alloc.dtype)))
                zero_outs.append(np.zeros(tuple(alloc.tensor_shape),
                                          mybir.dt.np(alloc.dtype)))
        self.in_names, self.out_names = in_names, out_names
        self.out_avals, self.zero_outs = out_avals, zero_outs
        n_params, n_outs = len(in_names), len(out_avals)
        all_in = list(in_names) + list(out_names)
        if partition_name is not None:
            all_in.append(partition_name)

        def _body(*args):
            operands = list(args)
            if partition_name is not None:
                operands.append(partition_id_tensor())
            return tuple(_bass_exec_p.bind(
                *operands, out_avals=tuple(out_avals), in_names=tuple(all_in),
                out_names=tuple(out_names), lowering_input_output_aliases=(),
                sim_require_finite=False, sim_require_nnan=False, nc=nc))

        devices = jax.devices()[:n_cores]
        self.mesh = Mesh(np.asarray(devices), ("core",))
        in_specs = (PartitionSpec("core"),) * (n_params + n_outs)
        out_specs = (PartitionSpec("core"),) * n_outs
        self.sharded = jax.jit(
            shard_map(_body, mesh=self.mesh, in_specs=in_specs,
                      out_specs=out_specs, check_rep=False),
            donate_argnums=tuple(range(n_params, n_params + n_outs)),
            keep_unused=True)

    def run(self, in_maps):
        import numpy as np
        from jax.sharding import NamedSharding, PartitionSpec
        sharding = NamedSharding(self.mesh, PartitionSpec("core"))
        concat = [self.jax.device_put(
            np.concatenate([np.asarray(in_maps[c][n]) for c in range(self.n_cores)], axis=0),
            sharding) for n in self.in_names]
        zeros = [self.jax.device_put(
            np.zeros((self.n_cores * z.shape[0], *z.shape[1:]), z.dtype), sharding)
            for z in self.zero_outs]
        outs = self.sharded(*concat, *zeros)
        self.jax.block_until_ready(outs)
        return [
            {n: np.asarray(outs[i]).reshape(self.n_cores, *self.out_avals[i].shape)[c]
             for i, n in enumerate(self.out_names)}
            for c in range(self.n_cores)
        ]


_CACHE = {}


def kernel(x, edge_index, batch, Ws, bs, head_w, head_b):
    import hashlib
    ins_per_core, struct = _prep(x, edge_index, batch, Ws, bs, head_w, head_b)
    h = hashlib.sha1()
    h.update(np.ascontiguousarray(edge_index).tobytes())
    h.update(np.ascontiguousarray(batch).tobytes())
    key = h.hexdigest()
    if key not in _CACHE:
        nc = _build(struct)
        _CACHE[key] = _Runner(nc, NC)
        _CACHE["gcn"] = _CACHE[key]
    runner = _CACHE[key]
    results = runner.run(ins_per_core)
    out = np.zeros(G, np.float64)
    for c in range(NC):
        part = results[c]["out"].reshape(-1)
        g0 = int(struct["pooled_base"][c])
        w = min(POOLW, G - g0)
        out[g0:g0 + w] += part[:w]
    out += struct["head_b"]
    return out.astype(np.float32)

